# revision 10
# baseline (speedup 1.0000x reference)
"""Trainium2 Bass kernel for nn_ASTMiniGNN (2-layer GCN over dense adjacency).

Sharding: data-parallel over the batch dim B=8 -> one graph per NeuronCore.
Per core:
  - gather X = emb[type_ids]                       (indirect DMA gather)
  - build transposed adjacency adjT (fp8, exact {0,1,2}) in DRAM via
    memset + indirect DMA scatter of the edge list (+ identity diagonal),
    then load it SBUF-resident (16 MB)
  - deg = colsum(adjT) via ones-vector matmuls; dis = 1/sqrt(deg)
  - layer1/layer2: (S+I) @ (dis*X) as bf16 x fp8 matmuls streaming adjT
    through the PE; dis column scaling folded algebraically downstream
  - mean-pool + L2 normalize on-chip; output row [1, 128] per core
"""

import numpy as np

B, N, E = 8, 4096, 8190
V, H = 256, 128
NCH = N // 128          # 32 node chunks
NSP = N // 512          # 8 free-dim spans
SCW = 64                # edge-scatter ops: 64 ops x 128 idx/partition = 8192 >= 8190
PAD = N * N             # scratch element index for scatter padding

_compiled = None


def _build():
    import os
    from contextlib import ExitStack
    import concourse.bass as bass
    import concourse.bacc as bacc
    import concourse.tile as tile
    from concourse.tile import add_dep_helper
    from concourse import mybir
    from concourse.masks import make_identity

    dt = mybir.dt
    AF = mybir.ActivationFunctionType
    OP = mybir.AluOpType

    nc = bacc.Bacc(
        "TRN2",
        target_bir_lowering=False,
        debug=False,
        enable_asserts=False,
        num_devices=B,
    )

    t_xgidx = nc.dram_tensor("xg_idx", [128, NCH], dt.int32, kind="ExternalInput")
    t_scoff = nc.dram_tensor("sc_off", [128, SCW], dt.int32, kind="ExternalInput")
    t_scval = nc.dram_tensor("sc_val", [128, SCW], dt.float8e4, kind="ExternalInput")
    t_dvals = nc.dram_tensor("dvals", [128, NCH], dt.float8e4, kind="ExternalInput")
    t_emb = nc.dram_tensor("emb", [V, H], dt.float32, kind="ExternalInput")
    t_w1t = nc.dram_tensor("w1t", [H, H], dt.bfloat16, kind="ExternalInput")
    t_w2t = nc.dram_tensor("w2t", [H, H], dt.bfloat16, kind="ExternalInput")
    t_b1bc = nc.dram_tensor("b1bc", [128, H], dt.float32, kind="ExternalInput")
    t_b2c = nc.dram_tensor("b2c", [H, 1], dt.float32, kind="ExternalInput")
    t_out = nc.dram_tensor("out", [1, H], dt.float32, kind="ExternalOutput")
    dbg = bool(os.environ.get("DBG"))
    t_dbg = (
        nc.dram_tensor("dbg", [128, 544], dt.float32, kind="ExternalOutput")
        if dbg
        else None
    )

    with tile.TileContext(nc) as tc, ExitStack() as ctx:
        dram = ctx.enter_context(tc.tile_pool(name="dram", bufs=1, space="DRAM"))
        main = ctx.enter_context(tc.tile_pool(name="main", bufs=1))
        tmp = ctx.enter_context(tc.tile_pool(name="tmp", bufs=3))
        psp = ctx.enter_context(tc.tile_pool(name="ps", bufs=1, space="PSUM"))

        adjd = dram.tile([N * N + 128], dt.float8e4)
        disd = dram.tile([N], dt.float32)
        adj2d = adjd[: N * N].rearrange("(c p i) -> c p i", p=128, i=N)

        # SBUF-resident adjacency: 32 tiles x [128, 4096] fp8 = 128 KB/partition
        adjsb = [main.tile([128, N], dt.float8e4, tag=f"adjT{c}", name=f"adjT{c}") for c in range(NCH)]
        Y = main.tile([128, N], dt.bfloat16, tag="Y")        # lhsT L1 (dis*X)
        H1r = main.tile([128, N], dt.bfloat16, tag="H1r")    # lhsT L2
        U1 = main.tile([128, N], dt.bfloat16, tag="U1")      # (S+I)Y spans
        disb = main.tile([128, N], dt.bfloat16, tag="disb")   # dis bcast all parts
        disc = main.tile([128, NCH], dt.float32, tag="disc")  # dis col layout
        zer = main.tile([128, N], dt.float8e4, tag="zer")

        idx = main.tile([128, NCH], dt.int32, tag="idx")
        scof = main.tile([128, SCW], dt.int32, tag="scof")
        scv = main.tile([128, SCW], dt.float8e4, tag="scv")
        dvt = main.tile([128, NCH], dt.float8e4, tag="dvt")
        w1s = main.tile([128, H], dt.bfloat16, tag="w1s")
        w2s = main.tile([128, H], dt.bfloat16, tag="w2s")
        b1s = main.tile([128, H], dt.float32, tag="b1s")
        b2s = main.tile([128, 1], dt.float32, tag="b2s")
        onec = main.tile([128, 1], dt.bfloat16, tag="onec")
        one1 = main.tile([1, 128], dt.float32, tag="one1")
        idn = main.tile([128, 128], dt.bfloat16, tag="idn")
        idnf = main.tile([128, 128], dt.float32, tag="idnf")
        disr = main.tile([1, N], dt.float32, tag="disr")

        nc.sync.dma_start(idx[:], t_xgidx[:])
        nc.sync.dma_start(scof[:], t_scoff[:])
        nc.sync.dma_start(scv[:], t_scval[:])
        nc.sync.dma_start(dvt[:], t_dvals[:])
        nc.sync.dma_start(w1s[:], t_w1t[:])
        nc.sync.dma_start(w2s[:], t_w2t[:])
        nc.sync.dma_start(b1s[:], t_b1bc[:])
        nc.sync.dma_start(b2s[:], t_b2c[:])
        nc.vector.memset(zer[:], 0.0)
        nc.vector.memset(onec[:], 1.0)
        nc.vector.memset(one1[:], 1.0)
        make_identity(nc, idn[:])
        make_identity(nc, idnf[:])

        # ---- adjacency DRAM memset (HWDGE) ----
        mem_insts = []
        for c in range(NCH):
            mi = nc.sync.dma_start(out=adj2d[c], in_=zer[:])
            mem_insts.append(mi.ins)

        # ---- X gather (SWDGE queue, overlaps memset) + cast to bf16 ----
        for c in range(NCH):
            gx = tmp.tile([128, H], dt.float32, tag="gx")
            if os.environ.get("SKIP_GATHER"):
                nc.vector.memset(gx[:], 0.01)
            else:
                nc.gpsimd.indirect_dma_start(
                    out=gx[:],
                    out_offset=None,
                    in_=t_emb[:],
                    in_offset=bass.IndirectOffsetOnAxis(ap=idx[:, c : c + 1], axis=0),
                )
            nc.vector.tensor_copy(Y[:, c * H : (c + 1) * H], gx[:])

        # ---- edge + diagonal scatter into adjacency ----
        # diagonal: strided DMA (element d*(N+1)); SP-queue FIFO orders it
        # after the memsets and before the readbacks automatically
        diag_ap = bass.AP(adjd.tensor, 0, [[(N + 1) * NCH, 128], [N + 1, NCH]])
        nc.sync.dma_start(out=diag_ap, in_=dvt[:])

        sc_inst = None
        if not os.environ.get("SKIP_SCATTER"):
            adj2col = adjd[:].rearrange("(a b) -> a b", b=1)
            for k in range(SCW):
                sc = nc.gpsimd.indirect_dma_start(
                    out=adj2col,
                    out_offset=bass.IndirectOffsetOnAxis(ap=scof[:, k : k + 1], axis=0),
                    in_=scv[:, k : k + 1],
                    in_offset=None,
                )
                sc_inst = sc.ins
                add_dep_helper(sc_inst, mem_insts[-1], reason="scatter after memset")

        # ---- load adjT to SBUF; deg colsums ride along on the PE ----
        degp = [psp.tile([1, 512], dt.float32, tag=f"ps{s}", name=f"degp{s}") for s in range(NSP)]
        for c in range(NCH):
            ri = nc.sync.dma_start(adjsb[c][:], adj2d[c])
            if sc_inst is not None:
                add_dep_helper(ri.ins, sc_inst, reason="readback waits for scatter")
            for s in range(NSP):
                nc.tensor.matmul(
                    degp[s][:],
                    lhsT=onec[:],
                    rhs=adjsb[c][:, s * 512 : (s + 1) * 512],
                    start=(c == 0),
                    stop=(c == NCH - 1),
                )

        # ---- dis = 1/sqrt(deg) ----
        for s in range(NSP):
            dsp = disr[:, s * 512 : (s + 1) * 512]
            nc.scalar.activation(dsp, degp[s][:], AF.Sqrt)
            nc.vector.reciprocal(dsp, dsp)

        # dis broadcast across partitions via K=1 matmul
        for s in range(NSP):
            pb = psp.tile([128, 512], dt.float32, tag=f"ps{s}")
            nc.tensor.matmul(
                pb[:],
                lhsT=one1[:],
                rhs=disr[:, s * 512 : (s + 1) * 512],
                start=True,
                stop=True,
            )
            nc.vector.tensor_copy(disb[:, s * 512 : (s + 1) * 512], pb[:])

        # dis in column layout via DRAM round-trip
        nc.sync.dma_start(disd[:], disr[:])
        nc.sync.dma_start(disc[:], disd[:].rearrange("(c p) -> p c", p=128))

        # ---- Y = dis * X (in place, per chunk) ----
        for c in range(NCH):
            nc.vector.tensor_scalar(
                out=Y[:, c * H : (c + 1) * H],
                in0=Y[:, c * H : (c + 1) * H],
                scalar1=disc[:, c : c + 1],
                scalar2=None,
                op0=OP.mult,
            )

        # ---- Layer 1: U1 = (S+I) @ Y  (transposed: [h_in, i]) ----
        psL1 = [psp.tile([128, 512], dt.float32, tag=f"ps{s}", name=f"psL1_{s}") for s in range(NSP)]
        for c in range(NCH):
            for s in range(NSP):
                nc.tensor.matmul(
                    psL1[s][:],
                    lhsT=Y[:, c * H : (c + 1) * H],
                    rhs=adjsb[c][:, s * 512 : (s + 1) * 512],
                    start=(c == 0),
                    stop=(c == NCH - 1),
                )
        for s in range(NSP):
            nc.vector.tensor_copy(U1[:, s * 512 : (s + 1) * 512], psL1[s][:])

        # WU = w1 @ U1 (reuses Y's SBUF slot; Y is dead after the L1 matmuls)
        WU = main.tile([128, N], dt.bfloat16, tag="Y")
        for s in range(NSP):
            pw = psp.tile([128, 512], dt.float32, tag=f"ps{s}")
            nc.tensor.matmul(
                pw[:],
                lhsT=w1s[:],
                rhs=U1[:, s * 512 : (s + 1) * 512],
                start=True,
                stop=True,
            )
            nc.vector.tensor_copy(WU[:, s * 512 : (s + 1) * 512], pw[:])

        # H1row chunks: transpose WU, then dis_j*relu(dis_j*wu + b1), as L2 lhsT
        for c in range(NCH):
            pt = psp.tile([128, 128], dt.bfloat16, tag=f"ps{c % 4}")
            nc.tensor.transpose(pt[:], WU[:, c * H : (c + 1) * H], idn[:])
            t1 = tmp.tile([128, 128], dt.float32, tag="t1")
            nc.vector.scalar_tensor_tensor(
                out=t1[:],
                in0=pt[:],
                scalar=disc[:, c : c + 1],
                in1=b1s[:],
                op0=OP.mult,
                op1=OP.add,
            )
            nc.vector.tensor_scalar(
                out=H1r[:, c * H : (c + 1) * H],
                in0=t1[:],
                scalar1=0.0,
                scalar2=disc[:, c : c + 1],
                op0=OP.max,
                op1=OP.mult,
            )

        # ---- Layer 2: U2 = (S+I) @ H1r ----
        psL2 = [psp.tile([128, 512], dt.float32, tag=f"ps{s}", name=f"psL2_{s}") for s in range(NSP)]
        for c in range(NCH):
            for s in range(NSP):
                nc.tensor.matmul(
                    psL2[s][:],
                    lhsT=H1r[:, c * H : (c + 1) * H],
                    rhs=adjsb[c][:, s * 512 : (s + 1) * 512],
                    start=(c == 0),
                    stop=(c == NCH - 1),
                )
        U2 = main.tile([128, N], dt.bfloat16, tag="U1")
        for s in range(NSP):
            nc.vector.tensor_copy(U2[:, s * 512 : (s + 1) * 512], psL2[s][:])

        # z^T spans = w2 @ U2; weighted mean-pool with dis columns
        zacc = None
        for s in range(NSP):
            pz = psp.tile([128, 512], dt.float32, tag=f"ps{s}")
            nc.tensor.matmul(
                pz[:],
                lhsT=w2s[:],
                rhs=U2[:, s * 512 : (s + 1) * 512],
                start=True,
                stop=True,
            )
            znew = tmp.tile([128, 1], dt.float32, tag=f"za{s % 2}")
            nc.vector.tensor_mul(
                U2[:, s * 512 : (s + 1) * 512],
                pz[:],
                disb[:, s * 512 : (s + 1) * 512],
            )
            nc.vector.reduce_sum(
                znew[:],
                U2[:, s * 512 : (s + 1) * 512],
                axis=mybir.AxisListType.X,
            )
            if zacc is not None:
                nc.vector.tensor_add(znew[:], znew[:], zacc[:])
            zacc = znew

        # zbar = zacc + b2 ; transpose to a row; normalize
        zb = tmp.tile([128, 1], dt.float32, tag="zb")
        nc.vector.tensor_scalar(
            out=zb[:],
            in0=zacc[:],
            scalar1=1.0 / N,
            scalar2=b2s[:],
            op0=OP.mult,
            op1=OP.add,
        )
        pzt = psp.tile([1, 128], dt.float32, tag="ps0")
        nc.tensor.transpose(pzt[:], zb[:], idnf[:])
        zrow = tmp.tile([1, 128], dt.float32, tag="zrow")
        nc.vector.tensor_copy(zrow[:], pzt[:])
        sqs = tmp.tile([1, 128], dt.float32, tag="sqs")
        n2 = tmp.tile([1, 1], dt.float32, tag="n2")
        nc.vector.tensor_mul(sqs[:], zrow[:], zrow[:])
        nc.vector.reduce_sum(n2[:], sqs[:], axis=mybir.AxisListType.X)
        sn = tmp.tile([1, 1], dt.float32, tag="sn")
        nc.scalar.activation(sn[:], n2[:], AF.Sqrt)
        inv = tmp.tile([1, 1], dt.float32, tag="inv")
        nc.vector.reciprocal(inv[:], sn[:])
        orow = tmp.tile([1, 128], dt.float32, tag="orow")
        nc.vector.tensor_scalar(
            out=orow[:], in0=zrow[:], scalar1=inv[:], scalar2=None, op0=OP.mult
        )
        nc.sync.dma_start(t_out[:], orow[:])

        if dbg:
            dgt = tmp.tile([128, 544], dt.float32, tag="dgt")
            nc.vector.tensor_copy(dgt[:, :512], adjsb[0][:, :512])
            nc.vector.tensor_copy(dgt[:, 512:544], disc[:])
            nc.sync.dma_start(t_dbg[:], dgt[:])

    nc.finalize()
    return nc


def _prep_core(type_ids_b, edges_b, emb, w1, b1, w2, b2, f8, bf16):
    ids = np.asarray(type_ids_b).astype(np.int64)
    ed = np.asarray(edges_b).astype(np.int64)
    a, bb = ed[:, 0], ed[:, 1]

    xg = ids.reshape(NCH, 128).T.astype(np.int32).copy()  # [128, NCH]

    selfm = a == bb
    off_e = (bb * N + a)[~selfm]
    has_self = np.zeros(N, np.float32)
    has_self[a[selfm]] = 1.0
    dvals = (1.0 + has_self).astype(f8).reshape(128, NCH)  # [p, c] -> node p*NCH+c

    npad = 128 * SCW - off_e.shape[0]
    offs = np.concatenate([off_e, np.full(npad, PAD, np.int64)]).astype(np.int32)
    vals = np.concatenate(
        [np.ones(off_e.shape[0], np.float32), np.zeros(npad, np.float32)]
    )

    return {
        "xg_idx": xg,
        "sc_off": offs.reshape(SCW, 128).T.copy(),
        "sc_val": vals.astype(f8).reshape(SCW, 128).T.copy(),
        "dvals": dvals,
        "emb": np.asarray(emb, np.float32),
        "w1t": np.asarray(w1, np.float32).T.astype(bf16).copy(),
        "w2t": np.asarray(w2, np.float32).T.astype(bf16).copy(),
        "b1bc": np.tile(np.asarray(b1, np.float32)[None, :], (128, 1)),
        "b2c": np.asarray(b2, np.float32).reshape(H, 1),
    }


def kernel(type_ids, edges, emb, w1, b1, w2, b2):
    global _compiled
    import ml_dtypes
    from concourse.bass_utils import run_bass_kernel_spmd

    f8 = ml_dtypes.float8_e4m3
    bf16 = ml_dtypes.bfloat16

    if _compiled is None:
        _compiled = _build()

    in_maps = [
        _prep_core(type_ids[b], edges[b], emb, w1, b1, w2, b2, f8, bf16)
        for b in range(B)
    ]
    res = run_bass_kernel_spmd(_compiled, in_maps, list(range(B)))
    rows = [np.asarray(res.results[b]["out"]).reshape(H) for b in range(B)]
    return np.stack(rows).astype(np.float32)


# revision 11
# speedup vs baseline: 2.9643x; 2.9643x over previous
"""Trainium2 Bass kernel for nn_ASTMiniGNN (2-layer GCN over dense adjacency).

Sharding: data-parallel over the batch dim B=8 -> one graph per NeuronCore.
Per core:
  - gather X = emb[type_ids]                       (indirect DMA gather)
  - build transposed adjacency adjT (fp8, exact {0,1,2}) in DRAM via
    memset + indirect DMA scatter of the edge list (+ identity diagonal),
    then load it SBUF-resident (16 MB)
  - deg = colsum(adjT) via ones-vector matmuls; dis = 1/sqrt(deg)
  - layer1/layer2: (S+I) @ (dis*X) as bf16 x fp8 matmuls streaming adjT
    through the PE; dis column scaling folded algebraically downstream
  - mean-pool + L2 normalize on-chip; output row [1, 128] per core
"""

import numpy as np

B, N, E = 8, 4096, 8190
V, H = 256, 128
NCH = N // 128          # 32 node chunks
NSP = N // 512          # 8 free-dim spans
SCW = 64                # edge-scatter ops: 64 ops x 128 idx/partition = 8192 >= 8190
PAD = N * N             # scratch element index for scatter padding

_compiled = None


def _build():
    import os
    from contextlib import ExitStack
    import concourse.bass as bass
    import concourse.bacc as bacc
    import concourse.tile as tile
    from concourse.tile import add_dep_helper
    from concourse import mybir
    from concourse.masks import make_identity

    dt = mybir.dt
    AF = mybir.ActivationFunctionType
    OP = mybir.AluOpType

    nc = bacc.Bacc(
        "TRN2",
        target_bir_lowering=False,
        debug=False,
        enable_asserts=False,
        num_devices=B,
    )

    t_xgidx = nc.dram_tensor("xg_idx", [128, NCH], dt.int32, kind="ExternalInput")
    t_scoff = nc.dram_tensor("sc_off", [128, SCW], dt.int32, kind="ExternalInput")
    t_scval = nc.dram_tensor("sc_val", [128, SCW], dt.float8e4, kind="ExternalInput")
    t_dvals = nc.dram_tensor("dvals", [128, NCH], dt.float8e4, kind="ExternalInput")
    t_emb = nc.dram_tensor("emb", [V, H], dt.float32, kind="ExternalInput")
    t_w1t = nc.dram_tensor("w1t", [H, H], dt.bfloat16, kind="ExternalInput")
    t_w2t = nc.dram_tensor("w2t", [H, H], dt.bfloat16, kind="ExternalInput")
    t_b1bc = nc.dram_tensor("b1bc", [128, H], dt.float32, kind="ExternalInput")
    t_b2c = nc.dram_tensor("b2c", [H, 1], dt.float32, kind="ExternalInput")
    t_out = nc.dram_tensor("out", [1, H], dt.float32, kind="ExternalOutput")
    dbg = bool(os.environ.get("DBG"))
    t_dbg = (
        nc.dram_tensor("dbg", [128, 544], dt.float32, kind="ExternalOutput")
        if dbg
        else None
    )

    with tile.TileContext(nc) as tc, ExitStack() as ctx:
        dram = ctx.enter_context(tc.tile_pool(name="dram", bufs=1, space="DRAM"))
        main = ctx.enter_context(tc.tile_pool(name="main", bufs=1))
        tmp = ctx.enter_context(tc.tile_pool(name="tmp", bufs=3))
        psp = ctx.enter_context(tc.tile_pool(name="ps", bufs=1, space="PSUM"))

        adjd = dram.tile([N * N + 128], dt.float8e4)
        disd = dram.tile([N], dt.float32)
        adj2d = adjd[: N * N].rearrange("(c p i) -> c p i", p=128, i=N)

        # SBUF-resident adjacency: 32 tiles x [128, 4096] fp8 = 128 KB/partition
        adjsb = [main.tile([128, N], dt.float8e4, tag=f"adjT{c}", name=f"adjT{c}") for c in range(NCH)]
        Y = main.tile([128, N], dt.bfloat16, tag="Y")        # lhsT L1 (dis*X)
        H1r = main.tile([128, N], dt.bfloat16, tag="H1r")    # lhsT L2
        U1 = main.tile([128, N], dt.bfloat16, tag="U1")      # (S+I)Y spans
        disb = main.tile([128, N], dt.bfloat16, tag="disb")   # dis bcast all parts
        disc = main.tile([128, NCH], dt.float32, tag="disc")  # dis col layout
        zer = main.tile([128, N], dt.float8e4, tag="zer")

        idx = main.tile([128, NCH], dt.int32, tag="idx")
        scof = main.tile([128, SCW], dt.int32, tag="scof")
        scv = main.tile([128, SCW], dt.float8e4, tag="scv")
        dvt = main.tile([128, NCH], dt.float8e4, tag="dvt")
        w1s = main.tile([128, H], dt.bfloat16, tag="w1s")
        w2s = main.tile([128, H], dt.bfloat16, tag="w2s")
        b1s = main.tile([128, H], dt.float32, tag="b1s")
        b2s = main.tile([128, 1], dt.float32, tag="b2s")
        onec = main.tile([128, 1], dt.bfloat16, tag="onec")
        one1 = main.tile([1, 128], dt.float32, tag="one1")
        idn = main.tile([128, 128], dt.bfloat16, tag="idn")
        idnf = main.tile([128, 128], dt.float32, tag="idnf")
        disr = main.tile([1, N], dt.float32, tag="disr")

        nc.sync.dma_start(idx[:], t_xgidx[:])
        nc.sync.dma_start(scof[:], t_scoff[:])
        nc.sync.dma_start(scv[:], t_scval[:])
        nc.sync.dma_start(dvt[:], t_dvals[:])
        nc.sync.dma_start(w1s[:], t_w1t[:])
        nc.sync.dma_start(w2s[:], t_w2t[:])
        nc.sync.dma_start(b1s[:], t_b1bc[:])
        nc.sync.dma_start(b2s[:], t_b2c[:])
        nc.vector.memset(zer[:], 0.0)
        nc.vector.memset(onec[:], 1.0)
        nc.vector.memset(one1[:], 1.0)
        make_identity(nc, idn[:])
        make_identity(nc, idnf[:])

        # ---- adjacency DRAM memset (HWDGE) ----
        mem_insts = []
        for c in range(NCH):
            mi = nc.sync.dma_start(out=adj2d[c], in_=zer[:])
            mem_insts.append(mi.ins)

        # ---- X gather (SWDGE queue, overlaps memset) + cast to bf16 ----
        for c in range(NCH):
            gx = tmp.tile([128, H], dt.float32, tag="gx")
            if os.environ.get("SKIP_GATHER"):
                nc.vector.memset(gx[:], 0.01)
            else:
                nc.gpsimd.indirect_dma_start(
                    out=gx[:],
                    out_offset=None,
                    in_=t_emb[:],
                    in_offset=bass.IndirectOffsetOnAxis(ap=idx[:, c : c + 1], axis=0),
                )
            nc.vector.tensor_copy(Y[:, c * H : (c + 1) * H], gx[:])

        # ---- edge + diagonal scatter into adjacency ----
        # diagonal: strided DMA (element d*(N+1)); SP-queue FIFO orders it
        # after the memsets and before the readbacks automatically
        diag_ap = bass.AP(adjd.tensor, 0, [[(N + 1) * NCH, 128], [N + 1, NCH]])
        nc.sync.dma_start(out=diag_ap, in_=dvt[:])

        sc_inst = None
        if not os.environ.get("SKIP_SCATTER"):
            adj2col = adjd[:].rearrange("(a b) -> a b", b=1)
            for k in range(SCW):
                sc = nc.gpsimd.indirect_dma_start(
                    out=adj2col,
                    out_offset=bass.IndirectOffsetOnAxis(ap=scof[:, k : k + 1], axis=0),
                    in_=scv[:, k : k + 1],
                    in_offset=None,
                )
                sc_inst = sc.ins
                add_dep_helper(sc_inst, mem_insts[-1], reason="scatter after memset")

        # ---- load adjT to SBUF; deg colsums ride along on the PE ----
        degp = [psp.tile([1, 512], dt.float32, tag=f"ps{s}", name=f"degp{s}") for s in range(NSP)]
        for c in range(NCH):
            ri = nc.sync.dma_start(adjsb[c][:], adj2d[c])
            if sc_inst is not None:
                add_dep_helper(ri.ins, sc_inst, reason="readback waits for scatter")
            for s in range(NSP):
                nc.tensor.matmul(
                    degp[s][:],
                    lhsT=onec[:],
                    rhs=adjsb[c][:, s * 512 : (s + 1) * 512],
                    start=(c == 0),
                    stop=(c == NCH - 1),
                )

        # ---- dis = 1/sqrt(deg) ----
        for s in range(NSP):
            dsp = disr[:, s * 512 : (s + 1) * 512]
            nc.scalar.activation(dsp, degp[s][:], AF.Sqrt)
            nc.vector.reciprocal(dsp, dsp)

        # dis broadcast across partitions via K=1 matmul
        for s in range(NSP):
            pb = psp.tile([128, 512], dt.float32, tag=f"ps{s}")
            nc.tensor.matmul(
                pb[:],
                lhsT=one1[:],
                rhs=disr[:, s * 512 : (s + 1) * 512],
                start=True,
                stop=True,
            )
            nc.vector.tensor_copy(disb[:, s * 512 : (s + 1) * 512], pb[:])

        # dis in column layout via DRAM round-trip
        nc.sync.dma_start(disd[:], disr[:])
        nc.sync.dma_start(disc[:], disd[:].rearrange("(c p) -> p c", p=128))

        # ---- Y = dis * X (in place, per chunk) ----
        for c in range(NCH):
            nc.vector.tensor_scalar(
                out=Y[:, c * H : (c + 1) * H],
                in0=Y[:, c * H : (c + 1) * H],
                scalar1=disc[:, c : c + 1],
                scalar2=None,
                op0=OP.mult,
            )

        # ---- Layer 1: U1 = (S+I) @ Y  (transposed: [h_in, i]) ----
        psL1 = [psp.tile([128, 512], dt.float32, tag=f"ps{s}", name=f"psL1_{s}") for s in range(NSP)]
        for c in range(NCH):
            for s in range(NSP):
                nc.tensor.matmul(
                    psL1[s][:],
                    lhsT=Y[:, c * H : (c + 1) * H],
                    rhs=adjsb[c][:, s * 512 : (s + 1) * 512],
                    start=(c == 0),
                    stop=(c == NCH - 1),
                )
        for s in range(NSP):
            nc.vector.tensor_copy(U1[:, s * 512 : (s + 1) * 512], psL1[s][:])

        # WU = w1 @ U1 (reuses Y's SBUF slot; Y is dead after the L1 matmuls)
        WU = main.tile([128, N], dt.bfloat16, tag="Y")
        for s in range(NSP):
            pw = psp.tile([128, 512], dt.float32, tag=f"ps{s}")
            nc.tensor.matmul(
                pw[:],
                lhsT=w1s[:],
                rhs=U1[:, s * 512 : (s + 1) * 512],
                start=True,
                stop=True,
            )
            nc.vector.tensor_copy(WU[:, s * 512 : (s + 1) * 512], pw[:])

        # H1row chunks: transpose WU, then dis_j*relu(dis_j*wu + b1), as L2 lhsT
        for c in range(NCH):
            pt = psp.tile([128, 128], dt.bfloat16, tag=f"ps{c % 4}")
            nc.tensor.transpose(pt[:], WU[:, c * H : (c + 1) * H], idn[:])
            t1 = tmp.tile([128, 128], dt.float32, tag="t1")
            nc.vector.scalar_tensor_tensor(
                out=t1[:],
                in0=pt[:],
                scalar=disc[:, c : c + 1],
                in1=b1s[:],
                op0=OP.mult,
                op1=OP.add,
            )
            nc.vector.tensor_scalar(
                out=H1r[:, c * H : (c + 1) * H],
                in0=t1[:],
                scalar1=0.0,
                scalar2=disc[:, c : c + 1],
                op0=OP.max,
                op1=OP.mult,
            )

        # ---- Layer 2: U2 = (S+I) @ H1r ----
        psL2 = [psp.tile([128, 512], dt.float32, tag=f"ps{s}", name=f"psL2_{s}") for s in range(NSP)]
        for c in range(NCH):
            for s in range(NSP):
                nc.tensor.matmul(
                    psL2[s][:],
                    lhsT=H1r[:, c * H : (c + 1) * H],
                    rhs=adjsb[c][:, s * 512 : (s + 1) * 512],
                    start=(c == 0),
                    stop=(c == NCH - 1),
                )
        U2 = main.tile([128, N], dt.bfloat16, tag="U1")
        for s in range(NSP):
            nc.vector.tensor_copy(U2[:, s * 512 : (s + 1) * 512], psL2[s][:])

        # z^T spans = w2 @ U2; weighted mean-pool with dis columns
        zacc = None
        for s in range(NSP):
            pz = psp.tile([128, 512], dt.float32, tag=f"ps{s}")
            nc.tensor.matmul(
                pz[:],
                lhsT=w2s[:],
                rhs=U2[:, s * 512 : (s + 1) * 512],
                start=True,
                stop=True,
            )
            znew = tmp.tile([128, 1], dt.float32, tag=f"za{s % 2}")
            nc.vector.tensor_mul(
                U2[:, s * 512 : (s + 1) * 512],
                pz[:],
                disb[:, s * 512 : (s + 1) * 512],
            )
            nc.vector.reduce_sum(
                znew[:],
                U2[:, s * 512 : (s + 1) * 512],
                axis=mybir.AxisListType.X,
            )
            if zacc is not None:
                nc.vector.tensor_add(znew[:], znew[:], zacc[:])
            zacc = znew

        # zbar = zacc + b2 ; transpose to a row; normalize
        zb = tmp.tile([128, 1], dt.float32, tag="zb")
        nc.vector.tensor_scalar(
            out=zb[:],
            in0=zacc[:],
            scalar1=1.0 / N,
            scalar2=b2s[:],
            op0=OP.mult,
            op1=OP.add,
        )
        pzt = psp.tile([1, 128], dt.float32, tag="ps0")
        nc.tensor.transpose(pzt[:], zb[:], idnf[:])
        zrow = tmp.tile([1, 128], dt.float32, tag="zrow")
        nc.vector.tensor_copy(zrow[:], pzt[:])
        sqs = tmp.tile([1, 128], dt.float32, tag="sqs")
        n2 = tmp.tile([1, 1], dt.float32, tag="n2")
        nc.vector.tensor_mul(sqs[:], zrow[:], zrow[:])
        nc.vector.reduce_sum(n2[:], sqs[:], axis=mybir.AxisListType.X)
        sn = tmp.tile([1, 1], dt.float32, tag="sn")
        nc.scalar.activation(sn[:], n2[:], AF.Sqrt)
        inv = tmp.tile([1, 1], dt.float32, tag="inv")
        nc.vector.reciprocal(inv[:], sn[:])
        orow = tmp.tile([1, 128], dt.float32, tag="orow")
        nc.vector.tensor_scalar(
            out=orow[:], in0=zrow[:], scalar1=inv[:], scalar2=None, op0=OP.mult
        )
        nc.sync.dma_start(t_out[:], orow[:])

        if dbg:
            dgt = tmp.tile([128, 544], dt.float32, tag="dgt")
            nc.vector.tensor_copy(dgt[:, :512], adjsb[0][:, :512])
            nc.vector.tensor_copy(dgt[:, 512:544], disc[:])
            nc.sync.dma_start(t_dbg[:], dgt[:])

    nc.finalize()
    return nc


def _prep_core(type_ids_b, edges_b, emb, w1, b1, w2, b2, f8, bf16):
    ids = np.asarray(type_ids_b).astype(np.int64)
    ed = np.asarray(edges_b).astype(np.int64)
    a, bb = ed[:, 0], ed[:, 1]

    xg = ids.reshape(NCH, 128).T.astype(np.int32).copy()  # [128, NCH]

    selfm = a == bb
    off_e = (bb * N + a)[~selfm]
    has_self = np.zeros(N, np.float32)
    has_self[a[selfm]] = 1.0
    dvals = (1.0 + has_self).astype(f8).reshape(128, NCH)  # [p, c] -> node p*NCH+c

    npad = 128 * SCW - off_e.shape[0]
    offs = np.concatenate([off_e, np.full(npad, PAD, np.int64)]).astype(np.int32)
    vals = np.concatenate(
        [np.ones(off_e.shape[0], np.float32), np.zeros(npad, np.float32)]
    )

    return {
        "xg_idx": xg,
        "sc_off": offs.reshape(SCW, 128).T.copy(),
        "sc_val": vals.astype(f8).reshape(SCW, 128).T.copy(),
        "dvals": dvals,
        "emb": np.asarray(emb, np.float32),
        "w1t": np.asarray(w1, np.float32).T.astype(bf16).copy(),
        "w2t": np.asarray(w2, np.float32).T.astype(bf16).copy(),
        "b1bc": np.tile(np.asarray(b1, np.float32)[None, :], (128, 1)),
        "b2c": np.asarray(b2, np.float32).reshape(H, 1),
    }


def kernel(type_ids, edges, emb, w1, b1, w2, b2):
    global _compiled
    import ml_dtypes
    from concourse.bass_utils import run_bass_kernel_spmd

    f8 = ml_dtypes.float8_e4m3
    bf16 = ml_dtypes.bfloat16

    if _compiled is None:
        _compiled = _build()

    in_maps = [
        _prep_core(type_ids[b], edges[b], emb, w1, b1, w2, b2, f8, bf16)
        for b in range(B)
    ]
    res = run_bass_kernel_spmd(_compiled, in_maps, list(range(B)))
    rows = [np.asarray(res.results[b]["out"]).reshape(H) for b in range(B)]
    return np.stack(rows).astype(np.float32)


def profile_once(inputs):
    """Run once with NTFF tracing; returns HW exec time in ns."""
    global _compiled
    import ml_dtypes, shutil, os
    from concourse.bass_utils import run_bass_kernel_spmd

    f8 = ml_dtypes.float8_e4m3
    bf16 = ml_dtypes.bfloat16
    if _compiled is None:
        _compiled = _build()
    in_maps = [
        _prep_core(
            inputs["type_ids"][b], inputs["edges"][b], inputs["emb"],
            inputs["w1"], inputs["b1"], inputs["w2"], inputs["b2"], f8, bf16,
        )
        for b in range(B)
    ]
    td = "/tmp/gnn_prof"
    shutil.rmtree(td, ignore_errors=True)
    os.makedirs(td, exist_ok=True)
    res = run_bass_kernel_spmd(
        _compiled, in_maps, list(range(B)), trace=True, tmpdir=td
    )
    return res.exec_time_ns
